# revision 14
# baseline (speedup 1.0000x reference)
"""Trainium2 Bass kernel for nn_DialogueGCNModel (DialogueGCN forward).

Strategy (data-parallel over dialogues, 4 dialogues per core):
  - Edges never cross dialogues, so the RGCN scatter/gather is dense
    per-dialogue banded-adjacency matmuls with masks in fp8.
  - All large matmuls run in fp8(e4m3) DoubleRow perf mode: K=256 per
    instruction at 0.5 cycles/row -> 4x PE throughput vs bf16, validated
    to ~5e-4 final rel err (gate 2e-2). Scales are folded into weights
    host-side and into activation scale/bias device-side.
  - The source-speaker factor of the relation masks is folded into the
    xr psum->sbuf copies (per-partition scalar multiply, free), and the
    1/deg normalization is baked into the mask values: masks shrink from
    (R=8) to (b,dir)=4 planes -> half the DMA bytes, no invd pass.
  - b_t is folded into the stage-5 matmul via a constant-one feature row
    (out2 pad subtile partition 0), so Xc copies are pure scaled casts.
  - softmax(tanh(s)) and log_softmax skip max-subtraction (tanh-bounded
    scores / tiny logits); the Ln activation table is preloaded via a
    dummy op so the final log_softmax doesn't eat the 1.3us table load.
  - Inputs stream in first-use order as split DMAs; the back half is
    pipelined per dialogue so softmax latency hides under the next
    dialogue's matmuls.

kernel(**inputs) takes FULL inputs, runs 8-core SPMD via
bass_utils.run_bass_kernel_spmd, returns the FULL (8192, 7) f32 output.
"""

import math

import numpy as np
import ml_dtypes

BF16 = ml_dtypes.bfloat16
FP8 = ml_dtypes.float8_e4m3

# Problem constants (hardcoded per contract)
B, L, D, H, R, NB, C = 32, 256, 1024, 128, 8, 30, 7
MEM = D + H            # 1152
N = B * L              # 8192
NCORES = 8
DPC = B // NCORES      # dialogues per core = 4
NLOC = DPC * L         # nodes per core = 1024
NT = NLOC // 128       # node tiles per core = 8
KT = D // 128          # contraction tiles over D = 8
MT = MEM // 128        # tiles over MEM = 9
MTP = MT + 1           # padded to even ktiles for DoubleRow pairing

# fp8 scale plan (host-folded; see prep_inputs)
S_WREL = 256.0
S_XR = 16.0
S_WR1 = 32.0           # out1T carries x32
S_O1 = 2.0
S_WT = 32.0
S_XC = 2.0
S_AL = 32.0            # exp output scale (normalization brings it to 64)
S_WL = 32.0
S_G = 2.0

_cache = {}


def _build_program(use_mask):
    import concourse.bacc as bacc
    import concourse.tile as tile
    import concourse.mybir as mybir
    import concourse.bass as bass
    from concourse.masks import make_identity

    dt = mybir.dt
    f32, bf16, fp8 = dt.float32, dt.bfloat16, dt.float8e4
    AX = mybir.AxisListType.X
    AF = mybir.ActivationFunctionType
    OP = mybir.AluOpType
    DR = mybir.MatmulPerfMode.DoubleRow

    nc = bacc.Bacc("TRN2", target_bir_lowering=False, debug=False,
                   num_devices=NCORES)

    dram = nc.dram_tensor
    xt_d = dram("xt", [D, NLOC], fp8, kind="ExternalInput")         # x^T
    wrel_d = dram("wrel", [D, R * H], fp8, kind="ExternalInput")
    wr1_d = dram("wr1", [D, H], fp8, kind="ExternalInput")
    spk_d = dram("spk", [NLOC, 2], f32, kind="ExternalInput")       # indicator/16
    at_d = dram("at", [DPC, 4, 128, 2, L], fp8, kind="ExternalInput")  # 2/deg
    bt_d = dram("bt", [DPC, 128, 2, L], fp8, kind="ExternalInput")
    w2_d = dram("w2", [2, H, H], bf16, kind="ExternalInput")
    wt_d = dram("wt", [MTP * 128, MEM], fp8, kind="ExternalInput")
    wlin_d = dram("wlin", [MTP * 128, H], fp8, kind="ExternalInput")
    wfc_d = dram("wfc", [H, C], bf16, kind="ExternalInput")
    bias_d = dram("bias", [128, 4], f32, kind="ExternalInput")
    bfc_d = dram("bfc", [1, 8], f32, kind="ExternalInput")
    if use_mask:
        um_d = dram("um", [DPC, 2, L], f32, kind="ExternalInput")
    out_d = dram("out", [NLOC, C], f32, kind="ExternalOutput")

    with tile.TileContext(nc) as tc:
        from contextlib import ExitStack
        with ExitStack() as ctx:
            consts = ctx.enter_context(tc.tile_pool(name="consts", bufs=1))
            big = ctx.enter_context(tc.tile_pool(name="big", bufs=1))
            work = ctx.enter_context(tc.tile_pool(name="work", bufs=6))
            ps = ctx.enter_context(tc.tile_pool(name="ps", bufs=6, space="PSUM"))
            pst = ctx.enter_context(tc.tile_pool(name="pst", bufs=2, space="PSUM"))

            mm = nc.tensor.matmul
            dma_sp = nc.sync.dma_start
            dma_gp = nc.gpsimd.dma_start

            # ---------------- persistent input loads (first-use order) ----
            wrel = consts.tile([128, KT, R * H], fp8)
            dma_sp(out=wrel, in_=wrel_d[:].rearrange("(k p) n -> p k n", p=128))
            xt = consts.tile([128, KT, NLOC], fp8)
            dma_sp(out=xt[:, :, 0:512],
                   in_=xt_d[:, 0:512].rearrange("(k p) n -> p k n", p=128))
            dma_sp(out=xt[:, :, 512:NLOC],
                   in_=xt_d[:, 512:NLOC].rearrange("(k p) n -> p k n", p=128))
            wt = consts.tile([128, MTP, MEM], fp8)
            dma_sp(out=wt[:, :, 0:576],
                   in_=wt_d[:, 0:576].rearrange("(m p) n -> p m n", p=128))
            dma_sp(out=wt[:, :, 576:MEM],
                   in_=wt_d[:, 576:MEM].rearrange("(m p) n -> p m n", p=128))
            wr1 = consts.tile([128, KT, H], fp8)
            dma_sp(out=wr1, in_=wr1_d[:].rearrange("(k p) n -> p k n", p=128))
            spk = consts.tile([128, NT, 2], f32)
            dma_sp(out=spk, in_=spk_d[:].rearrange("(i p) a -> p i a", p=128))
            at = consts.tile([128, DPC, 4, 2, L], fp8)
            for d in range(DPC):
                dma_sp(out=at[:, d, :, :, :],
                       in_=at_d[d].rearrange("e p st t -> p e st t"))
            bt = consts.tile([128, DPC, 2, L], fp8)
            dma_sp(out=bt, in_=bt_d[:].rearrange("d p st t -> p d st t"))
            wlin = consts.tile([128, MTP, H], fp8)
            dma_sp(out=wlin, in_=wlin_d[:].rearrange("(m p) n -> p m n", p=128))
            wfc = consts.tile([128, C], bf16)
            dma_sp(out=wfc, in_=wfc_d[:])
            bias = consts.tile([128, 4], f32)
            dma_sp(out=bias, in_=bias_d[:])
            w2 = consts.tile([128, 2, H], bf16)
            dma_gp(out=w2, in_=w2_d[:].rearrange("j p h -> p j h"))

            def bcast(dst, src_ap):
                bc = bass.AP(tensor=src_ap.tensor, offset=src_ap.offset,
                             ap=[[0, 128]] + list(src_ap.ap))
                nc.gpsimd.dma_start(out=dst, in_=bc)

            bfc = consts.tile([128, 8], f32)
            bcast(bfc, bfc_d[0, :])
            if use_mask:
                um = consts.tile([128, DPC, 2, L], f32)
                bcast(um, um_d[:])

            ident = consts.tile([128, 128], bf16)
            make_identity(nc, ident)

            # ---------------- persistent activation tiles ----------------
            xr = consts.tile([128, NT, R * H], fp8)       # 16*xr*spk_mask
            out1T = consts.tile([128, DPC, L], bf16)      # 32*out1^T
            out1 = consts.tile([128, NT, H], fp8)         # 2*out1
            o2p = consts.tile([128, DPC, 2, L], fp8)      # [out2; e0-ones pad]
            XcTs = [big.tile([128, MTP, L], fp8, tag=f"XcT{d}", name=f"XcT{d}")
                    for d in range(DPC)]                  # 2*Xc^T (plane 9 = 0)
            zs = [big.tile([128, 2, L], bf16, tag=f"z{d}", name=f"z{d}")
                  for d in range(DPC)]
            alfs = [big.tile([128, 2, L], bf16, tag=f"alf{d}", name=f"alf{d}")
                    for d in range(DPC)]
            alphaTs = [big.tile([128, 2, L], fp8, tag=f"alphaT{d}",
                                name=f"alphaT{d}") for d in range(DPC)]
            Gs = [big.tile([128, 2, H], fp8, tag=f"G{d}", name=f"G{d}")
                  for d in range(DPC)]
            hidT = consts.tile([128, DPC, L], bf16)
            o_all = consts.tile([128, 2 * DPC, 8], f32)
            e_all = consts.tile([128, 2 * DPC, 8], f32)
            s7p = consts.tile([128, 2 * DPC], f32)
            ls = consts.tile([128, 2 * DPC], f32)
            lnscr = consts.tile([128, 1], f32)
            ssums = [work.tile([128, 2], f32, tag=f"ssum{d}", name=f"ssum{d}")
                     for d in range(DPC)]
            rrs = [work.tile([128, 2], f32, tag=f"rr{d}", name=f"rr{d}")
                   for d in range(DPC)]

            # zero pads / constant-one feature row (Pool)
            nc.gpsimd.memset(o2p[:, :, 1, :], 0.0)
            nc.gpsimd.memset(o2p[0:1, :, 1, :], 1.0)
            for d in range(DPC):
                nc.gpsimd.memset(XcTs[d][:, MT, :], 0.0)
            nc.gpsimd.memset(o_all, 0.0)

            def bc_ap(t, ap):
                src = t[:]
                return bass.AP(tensor=src.tensor, offset=src.offset, ap=ap)

            # warm-up: hold the PE clock ramp during the DMA lead-in.
            warm_in = consts.tile([128, 128], bf16)
            nc.vector.memset(warm_in, 0.0)
            warm = ps.tile([128, 512], f32, tag="mm")
            for _ in range(85):
                mm(warm[:, :128], lhsT=warm_in, rhs=warm_in, start=True,
                   stop=True, skip_group_check=True)

            # round-robin copy-out engine picker (psum sources: DVE/ACT only)
            rr_state = [0]

            def copy_scaled(out, in_, scale, bias_col=None, eng=None):
                """out = in_*scale (+bias_col) on DVE/ACT ('v'/'a'),
                alternating if eng is None. scale: float or [128,1] AP."""
                if eng is None:
                    eng = 'v' if rr_state[0] % 2 == 0 else 'a'
                    rr_state[0] += 1
                if eng == 'v':
                    if bias_col is None:
                        nc.vector.tensor_scalar(out=out, in0=in_, scalar1=scale,
                                                scalar2=None, op0=OP.mult)
                    else:
                        nc.vector.tensor_scalar(out=out, in0=in_, scalar1=scale,
                                                scalar2=bias_col, op0=OP.mult,
                                                op1=OP.add)
                else:
                    nc.scalar.activation(out, in_, AF.Identity, scale=scale,
                                         bias=bias_col if bias_col is not None
                                         else 0.0)

            # ---------------- stage 1: xr = x @ w_rel ----------------
            # psum = 256*xr_half; xr stores 16*xr masked by src speaker
            for i in range(NT):
                for h2 in range(2):
                    p = ps.tile([128, 512], f32, tag="mm")
                    for kk in range(KT // 2):
                        mm(p, lhsT=xt[:, 2 * kk:2 * kk + 2, i * 128:(i + 1) * 128],
                           rhs=wrel[:, 2 * kk:2 * kk + 2, h2 * 512:(h2 + 1) * 512],
                           start=(kk == 0), stop=(kk == KT // 2 - 1), perf_mode=DR)
                    copy_scaled(xr[:, i, h2 * 512:(h2 + 1) * 512], p,
                                spk[:, i, h2:h2 + 1])

            # ---------------- stage 2: out1T = 32*(agg/deg + x@w_root1 + b1)
            # mask values are 2/deg; xr already speaker-masked
            for d in range(DPC):
                pa = ps.tile([128, 512], f32, tag="mm")
                for e in range(4):          # e = (dst_spk, dir)
                    for a in range(2):      # src speaker half
                        r = a * 4 + e
                        mm(pa[:, :L],
                           lhsT=xr[:, 2 * d:2 * d + 2, r * H:(r + 1) * H],
                           rhs=at[:, d, e, :, :], start=(e == 0 and a == 0),
                           stop=False, perf_mode=DR)
                for kk in range(KT // 2):   # root term: same psum group
                    mm(pa[:, :L], lhsT=wr1[:, 2 * kk:2 * kk + 2, :],
                       rhs=xt[:, 2 * kk:2 * kk + 2, d * L:(d + 1) * L],
                       start=False, stop=(kk == KT // 2 - 1), perf_mode=DR)
                nc.vector.tensor_scalar(out=out1T[:, d, :], in0=pa[:, :L],
                                        scalar1=bias[:, 0:1], scalar2=None,
                                        op0=OP.add)

            # out1 (fp8, 2*out1) via PE transpose
            for d in range(DPC):
                for st in range(2):
                    tp = pst.tile([128, 128], bf16, tag="tr")
                    nc.tensor.transpose(tp, out1T[:, d, st * 128:(st + 1) * 128],
                                        ident)
                    nc.vector.tensor_scalar(out=out1[:, 2 * d + st, :], in0=tp,
                                            scalar1=S_O1 / S_WR1, scalar2=None,
                                            op0=OP.mult)

            # ---------------- stage 3: GraphConv layer 2 ----------------
            nbTs = []
            for d in range(DPC):
                p2 = ps.tile([128, 512], f32, tag="mm")
                mm(p2[:, :L], lhsT=out1[:, 2 * d:2 * d + 2, :], rhs=bt[:, d, :, :],
                   start=True, stop=True, perf_mode=DR)
                nbT = work.tile([128, L], bf16, tag=f"nbT{d}", name=f"nbT{d}")
                copy_scaled(nbT, p2[:, :L], 1.0)
                nbTs.append(nbT)
            for d in range(DPC):
                p3 = ps.tile([128, 512], f32, tag="mm")
                mm(p3[:, :L], lhsT=w2[:, 0, :], rhs=nbTs[d], start=True, stop=False)
                mm(p3[:, :L], lhsT=w2[:, 1, :], rhs=out1T[:, d, :],
                   start=False, stop=True)
                copy_scaled(o2p[:, d, 0, :], p3[:, :L], 1.0, bias[:, 1:2])

            # M^T k-pair accessor: j in 0..4 -> [128, 2, cols] fp8
            def mpair(j, d, c0=0, c1=L):
                if j < 4:
                    return xt[:, 2 * j:2 * j + 2, d * L + c0:d * L + c1]
                return o2p[:, d, :, c0:c1]

            # ---------------- G = M @ w_lin (independent of stage 5/6) ----
            for d in range(DPC):
                for st in range(2):
                    pg = ps.tile([128, 512], f32, tag="mm")
                    for j in range(5):
                        mm(pg[:, :H], lhsT=mpair(j, d, st * 128, (st + 1) * 128),
                           rhs=wlin[:, 2 * j:2 * j + 2, :],
                           start=(j == 0), stop=(j == 4), perf_mode=DR)
                    copy_scaled(Gs[d][:, st, :], pg[:, :H], S_G / S_WL,
                                eng='v')

            # ------------- per-dialogue pipelined back half ---------------
            def s5(d):
                # Xc^T = 2*(M @ w_t + b_t) (b_t via ones-feature, wt plane 9)
                for n2 in range(MT):
                    p4 = ps.tile([128, 512], f32, tag="mm")
                    for j in range(5):
                        mm(p4[:, :L], lhsT=wt[:, 2 * j:2 * j + 2,
                                              n2 * 128:(n2 + 1) * 128],
                           rhs=mpair(j, d), start=(j == 0), stop=(j == 4),
                           perf_mode=DR)
                    copy_scaled(XcTs[d][:, n2, :], p4[:, :L], 1.0 / S_WT,
                                eng=('a' if n2 % 3 == 2 else 'v'))

            def s6(d):
                # scores -> tanh -> exp -> normalized 64*alpha (bf16)
                for tt in range(2):
                    p5 = ps.tile([128, 512], f32, tag="mm")
                    for j in range(5):
                        mm(p5[:, :L],
                           lhsT=XcTs[d][:, 2 * j:2 * j + 2,
                                        tt * 128:(tt + 1) * 128],
                           rhs=mpair(j, d), start=(j == 0), stop=(j == 4),
                           perf_mode=DR)
                    if use_mask:
                        zf = work.tile([128, L], f32, tag="zf", name="zf")
                        nc.vector.tensor_mul(zf, p5[:, :L], um[:, d, 0, :])
                        nc.scalar.activation(zs[d][:, tt, :], zf, AF.Tanh)
                    else:
                        nc.scalar.activation(zs[d][:, tt, :], p5[:, :L], AF.Tanh,
                                             scale=1.0 / S_XC)
                nc.scalar.activation(alfs[d], zs[d], AF.Exp, bias=bias[:, 3:4])
                if use_mask:
                    for tt in range(2):
                        nc.gpsimd.tensor_mul(alfs[d][:, tt, :],
                                             alfs[d][:, tt, :], um[:, d, 1, :])
                for tt in range(2):
                    nc.vector.reduce_sum(out=ssums[d][:, tt:tt + 1],
                                         in_=alfs[d][:, tt, :], axis=AX)
                nc.vector.reciprocal(rrs[d], ssums[d])
                nc.gpsimd.tensor_scalar_mul(rrs[d], rrs[d], 64.0)
                nc.gpsimd.tensor_tensor(
                    out=alfs[d], in0=alfs[d],
                    in1=bc_ap(rrs[d], list(rrs[d][:].ap[:-1])
                              + [list(rrs[d][:].ap[-1]), [0, L]]),
                    op=OP.mult)

            def att_cls(d):
                # alpha^T (PE transpose) -> hidden^T -> logits -> o_all
                for tt in range(2):
                    for st in range(2):
                        tp = pst.tile([128, 128], bf16, tag="tr")
                        nc.tensor.transpose(
                            tp, alfs[d][:, tt, st * 128:(st + 1) * 128], ident)
                        copy_scaled(alphaTs[d][:, st, tt * 128:(tt + 1) * 128],
                                    tp, 1.0)
                p7 = ps.tile([128, 512], f32, tag="mm")
                mm(p7[:, :L], lhsT=Gs[d][:, :, :], rhs=alphaTs[d][:, :, :],
                   start=True, stop=True, perf_mode=DR)
                nc.scalar.activation(hidT[:, d, :], p7[:, :L], AF.Relu,
                                     scale=1.0 / (S_G * 64.0), bias=bias[:, 2:3])
                for tt in range(2):
                    g = 2 * d + tt
                    p8 = ps.tile([128, 512], f32, tag="mm")
                    mm(p8[:, :C], lhsT=hidT[:, d, tt * 128:(tt + 1) * 128],
                       rhs=wfc, start=True, stop=True)
                    nc.vector.tensor_tensor(out=o_all[:, g, 0:7], in0=p8[:, :C],
                                            in1=bfc[:, 0:7], op=OP.add)

            # schedule: softmax(d) latency hides under s5/s6 of d+1
            s5(0); s6(0)
            s5(1); s6(1)
            att_cls(0)
            s5(2); s6(2)
            att_cls(1)
            s5(3); s6(3)
            # preload the natural_log_exp ACT table (covers exp/relu/identity
            # for everything after the last tanh; Ln then loads no table)
            nc.scalar.activation(lnscr, bias[:, 3:4], AF.Ln)
            att_cls(2)
            att_cls(3)

            # log_softmax over C=7 (logits are tiny: skip max-sub)
            nc.scalar.activation(e_all, o_all, AF.Exp)
            nc.vector.reduce_sum(out=s7p, in_=e_all[:, :, 0:7], axis=AX)
            nc.scalar.activation(ls, s7p, AF.Ln)
            nc.vector.tensor_tensor(
                out=o_all[:, :, 0:7], in0=o_all[:, :, 0:7],
                in1=bc_ap(ls, [list(ls[:].ap[0]), list(ls[:].ap[1]), [0, 7]]),
                op=OP.subtract)
            dma_sp(out=out_d[:].rearrange("(g p) c -> p g c", p=128),
                   in_=o_all[:, :, 0:7])

    return _finish(nc)


def _finish(nc):
    nc.compile()
    return nc


def prep_inputs(x, edge_src, edge_dst, edge_type, umask, basis, comp,
                w_root1, b1, w_rel2, b_rel2, w_root2, w_t, b_t,
                w_lin, b_lin, w_fc, b_fc):
    """Host-side sharding / layout / fp8-scale prep."""
    x = np.asarray(x, np.float32)
    src = np.asarray(edge_src, np.int64)
    dst = np.asarray(edge_dst, np.int64)
    ety = np.asarray(edge_type, np.int64)
    umask = np.asarray(umask, np.float32)

    g_s = src // L
    assert np.array_equal(g_s, dst // L), "edges must stay within a dialogue"

    def q8(a, scale):
        return np.clip(np.asarray(a, np.float32) * scale,
                       -240.0, 240.0).astype(FP8)

    w_rel = np.einsum('rb,bdh->rdh', np.asarray(comp, np.float32),
                      np.asarray(basis, np.float32))
    wrel_in = q8(np.ascontiguousarray(
        w_rel.transpose(1, 0, 2).reshape(D, R * H)), S_WREL)

    deg = np.bincount(dst, minlength=N).astype(np.float64)
    inv2 = (2.0 / np.maximum(deg, 1)) * (deg > 0)

    # speaker of each node from edge types (etype = a*4 + b*2 + dir)
    spk_node = np.zeros(N, np.int64)
    spk_node[src] = ety // 4
    spk_in = np.zeros((N, 2), np.float32)
    spk_in[np.arange(N), spk_node] = 1.0 / S_XR

    # (b, dir) masks with 2/deg baked into the values
    at_all = np.zeros((B, 4, L, L), np.float32)
    np.add.at(at_all, (g_s, ety % 4, src % L, dst % L), inv2[dst])
    bt_all = np.zeros((B, L, L), np.float32)
    np.add.at(bt_all, (g_s, src % L, dst % L), 1.0)
    # p-major layouts with >=512B contiguous inner runs
    at_all = at_all.reshape(B, 4, 2, 128, L).transpose(0, 1, 3, 2, 4)
    bt_all = bt_all.reshape(B, 2, 128, L).transpose(0, 2, 1, 3)

    use_mask = not bool(np.all(umask == 1.0))

    wt_in = np.zeros((MTP * 128, MEM), np.float32)
    wt_in[:MEM] = np.asarray(w_t, np.float32) * S_WT
    wt_in[MEM] = np.asarray(b_t, np.float32) * S_WT    # ones-feature row
    wlin_in = np.zeros((MTP * 128, H), np.float32)
    wlin_in[:MEM] = np.asarray(w_lin, np.float32) * S_WL

    bias_pack = np.zeros((128, 4), np.float32)
    bias_pack[:, 0] = np.asarray(b1, np.float32) * S_WR1
    bias_pack[:, 1] = np.asarray(b_rel2, np.float32)
    bias_pack[:, 2] = np.asarray(b_lin, np.float32)
    bias_pack[:, 3] = math.log(S_AL)

    bfc_in = np.zeros((1, 8), np.float32)
    bfc_in[0, :C] = np.asarray(b_fc, np.float32)

    shared = {
        "wrel": wrel_in,
        "wr1": q8(w_root1, S_WR1),
        "w2": np.stack([np.asarray(w_rel2, np.float32) / S_O1,
                        np.asarray(w_root2, np.float32) / S_WR1]).astype(BF16),
        "wt": np.clip(wt_in, -240, 240).astype(FP8),
        "wlin": np.clip(wlin_in, -240, 240).astype(FP8),
        "wfc": np.asarray(w_fc, np.float32).astype(BF16),
        "bias": bias_pack,
        "bfc": bfc_in,
    }

    in_maps = []
    for c in range(NCORES):
        xl = x[c * NLOC:(c + 1) * NLOC]
        m = dict(shared)
        m["xt"] = q8(np.ascontiguousarray(xl.T), 1.0)
        m["spk"] = spk_in[c * NLOC:(c + 1) * NLOC]
        m["at"] = at_all[c * DPC:(c + 1) * DPC].astype(FP8)
        m["bt"] = bt_all[c * DPC:(c + 1) * DPC].astype(FP8)
        if use_mask:
            uml = umask[c * DPC:(c + 1) * DPC]   # (DPC, L)
            m["um"] = np.stack([uml * uml / S_XC, uml], axis=1
                               ).astype(np.float32)
        in_maps.append(m)
    return in_maps, use_mask


_last_results = None


def kernel(**inputs):
    global _last_results
    from concourse.bass_utils import run_bass_kernel_spmd

    in_maps, use_mask = prep_inputs(**inputs)
    if use_mask not in _cache:
        _cache[use_mask] = _build_program(use_mask)
    nc = _cache[use_mask]
    res = run_bass_kernel_spmd(nc, in_maps, core_ids=list(range(NCORES)))
    _last_results = res
    return np.concatenate([res.results[c]["out"] for c in range(NCORES)],
                          axis=0)


# revision 15
# speedup vs baseline: 1.0965x; 1.0965x over previous
"""Trainium2 Bass kernel for nn_DialogueGCNModel (DialogueGCN forward).

Strategy (data-parallel over dialogues, 4 dialogues per core):
  - Edges never cross dialogues, so the RGCN scatter/gather is dense
    per-dialogue banded-adjacency matmuls with masks in fp8.
  - All large matmuls run in fp8(e4m3) DoubleRow perf mode: K=256 per
    instruction at 0.5 cycles/row -> 4x PE throughput vs bf16, validated
    to ~5e-4 final rel err (gate 2e-2). Scales are folded into weights
    host-side and into activation scale/bias device-side.
  - The source-speaker factor of the relation masks is folded into the
    xr psum->sbuf copies (per-partition scalar multiply, free), and the
    1/deg normalization is baked into the mask values: masks shrink from
    (R=8) to (b,dir)=4 planes -> half the DMA bytes, no invd pass.
  - b_t is folded into the stage-5 matmul via a constant-one feature row
    (out2 pad subtile partition 0), so Xc copies are pure scaled casts.
  - softmax(tanh(s)) and log_softmax skip max-subtraction (tanh-bounded
    scores / tiny logits); the Ln activation table is preloaded via a
    dummy op so the final log_softmax doesn't eat the 1.3us table load.
  - Inputs stream in first-use order as split DMAs; the back half is
    pipelined per dialogue so softmax latency hides under the next
    dialogue's matmuls.

kernel(**inputs) takes FULL inputs, runs 8-core SPMD via
bass_utils.run_bass_kernel_spmd, returns the FULL (8192, 7) f32 output.
"""

import math

import numpy as np
import ml_dtypes

BF16 = ml_dtypes.bfloat16
FP8 = ml_dtypes.float8_e4m3

# Problem constants (hardcoded per contract)
B, L, D, H, R, NB, C = 32, 256, 1024, 128, 8, 30, 7
MEM = D + H            # 1152
N = B * L              # 8192
NCORES = 8
DPC = B // NCORES      # dialogues per core = 4
NLOC = DPC * L         # nodes per core = 1024
NT = NLOC // 128       # node tiles per core = 8
KT = D // 128          # contraction tiles over D = 8
MT = MEM // 128        # tiles over MEM = 9
MTP = MT + 1           # padded to even ktiles for DoubleRow pairing

# fp8 scale plan (host-folded; see prep_inputs)
S_WREL = 256.0
S_XR = 16.0
S_WR1 = 32.0           # out1T carries x32
S_O1 = 2.0
S_WT = 32.0
S_XC = 2.0
S_AL = 32.0            # exp output scale (normalization brings it to 64)
S_WL = 32.0
S_G = 2.0

_cache = {}


def _build_program(use_mask):
    import concourse.bacc as bacc
    import concourse.tile as tile
    import concourse.mybir as mybir
    import concourse.bass as bass
    from concourse.masks import make_identity

    dt = mybir.dt
    f32, bf16, fp8 = dt.float32, dt.bfloat16, dt.float8e4
    AX = mybir.AxisListType.X
    AF = mybir.ActivationFunctionType
    OP = mybir.AluOpType
    DR = mybir.MatmulPerfMode.DoubleRow

    nc = bacc.Bacc("TRN2", target_bir_lowering=False, debug=False,
                   num_devices=NCORES)

    dram = nc.dram_tensor
    xt_d = dram("xt", [D, NLOC], fp8, kind="ExternalInput")         # x^T
    wrel_d = dram("wrel", [D, R * H], fp8, kind="ExternalInput")
    wr1_d = dram("wr1", [D, H], fp8, kind="ExternalInput")
    spk_d = dram("spk", [NLOC, 2], f32, kind="ExternalInput")       # indicator/16
    at_d = dram("at", [DPC, 4, 128, 2, L], fp8, kind="ExternalInput")  # 2/deg
    bt_d = dram("bt", [DPC, 128, 2, L], fp8, kind="ExternalInput")
    w2_d = dram("w2", [2, H, H], bf16, kind="ExternalInput")
    wt_d = dram("wt", [MTP * 128, MEM], fp8, kind="ExternalInput")
    wlin_d = dram("wlin", [MTP * 128, H], fp8, kind="ExternalInput")
    wfc_d = dram("wfc", [H, C], bf16, kind="ExternalInput")
    bias_d = dram("bias", [128, 4], f32, kind="ExternalInput")
    bfc_d = dram("bfc", [1, 8], f32, kind="ExternalInput")
    if use_mask:
        um_d = dram("um", [DPC, 2, L], f32, kind="ExternalInput")
    out_d = dram("out", [NLOC, C], f32, kind="ExternalOutput")

    with tile.TileContext(nc) as tc:
        from contextlib import ExitStack
        with ExitStack() as ctx:
            consts = ctx.enter_context(tc.tile_pool(name="consts", bufs=1))
            big = ctx.enter_context(tc.tile_pool(name="big", bufs=1))
            work = ctx.enter_context(tc.tile_pool(name="work", bufs=6))
            ps = ctx.enter_context(tc.tile_pool(name="ps", bufs=6, space="PSUM"))
            pst = ctx.enter_context(tc.tile_pool(name="pst", bufs=2, space="PSUM"))

            mm = nc.tensor.matmul
            dma_sp = nc.sync.dma_start
            dma_gp = nc.gpsimd.dma_start

            # ---------------- persistent input loads (first-use order) ----
            wrel = consts.tile([128, KT, R * H], fp8)
            dma_sp(out=wrel, in_=wrel_d[:].rearrange("(k p) n -> p k n", p=128))
            xt = consts.tile([128, KT, NLOC], fp8)
            dma_sp(out=xt[:, :, 0:512],
                   in_=xt_d[:, 0:512].rearrange("(k p) n -> p k n", p=128))
            dma_sp(out=xt[:, :, 512:NLOC],
                   in_=xt_d[:, 512:NLOC].rearrange("(k p) n -> p k n", p=128))
            wr1 = consts.tile([128, KT, H], fp8)
            dma_sp(out=wr1, in_=wr1_d[:].rearrange("(k p) n -> p k n", p=128))
            spk = consts.tile([128, NT, 2], f32)
            dma_sp(out=spk, in_=spk_d[:].rearrange("(i p) a -> p i a", p=128))
            at = consts.tile([128, DPC, 4, 2, L], fp8)
            for d in range(DPC):
                dma_sp(out=at[:, d, :, :, :],
                       in_=at_d[d].rearrange("e p st t -> p e st t"))
            bt = consts.tile([128, DPC, 2, L], fp8)
            dma_sp(out=bt, in_=bt_d[:].rearrange("d p st t -> p d st t"))
            wlin = consts.tile([128, MTP, H], fp8)
            dma_sp(out=wlin, in_=wlin_d[:].rearrange("(m p) n -> p m n", p=128))
            wt = consts.tile([128, MTP, MEM], fp8)
            dma_sp(out=wt[:, :, 0:576],
                   in_=wt_d[:, 0:576].rearrange("(m p) n -> p m n", p=128))
            dma_sp(out=wt[:, :, 576:MEM],
                   in_=wt_d[:, 576:MEM].rearrange("(m p) n -> p m n", p=128))
            wfc = consts.tile([128, C], bf16)
            dma_sp(out=wfc, in_=wfc_d[:])
            bias = consts.tile([128, 4], f32)
            dma_sp(out=bias, in_=bias_d[:])
            w2 = consts.tile([128, 2, H], bf16)
            dma_gp(out=w2, in_=w2_d[:].rearrange("j p h -> p j h"))

            def bcast(dst, src_ap):
                bc = bass.AP(tensor=src_ap.tensor, offset=src_ap.offset,
                             ap=[[0, 128]] + list(src_ap.ap))
                nc.gpsimd.dma_start(out=dst, in_=bc)

            bfc = consts.tile([128, 8], f32)
            bcast(bfc, bfc_d[0, :])
            if use_mask:
                um = consts.tile([128, DPC, 2, L], f32)
                bcast(um, um_d[:])

            ident = consts.tile([128, 128], bf16)
            make_identity(nc, ident)

            # ---------------- persistent activation tiles ----------------
            xr = consts.tile([128, NT, R * H], fp8)       # 16*xr*spk_mask
            out1T = consts.tile([128, DPC, L], bf16)      # 32*out1^T
            out1 = consts.tile([128, NT, H], fp8)         # 2*out1
            o2p = consts.tile([128, DPC, 2, L], fp8)      # [out2; e0-ones pad]
            XcTs = [big.tile([128, MTP, L], fp8, tag=f"XcT{d}", name=f"XcT{d}")
                    for d in range(DPC)]                  # 2*Xc^T (plane 9 = 0)
            zs = [big.tile([128, 2, L], bf16, tag=f"z{d}", name=f"z{d}")
                  for d in range(DPC)]
            alfs = [big.tile([128, 2, L], bf16, tag=f"alf{d}", name=f"alf{d}")
                    for d in range(DPC)]
            alphaTs = [big.tile([128, 2, L], fp8, tag=f"alphaT{d}",
                                name=f"alphaT{d}") for d in range(DPC)]
            Gs = [big.tile([128, 2, H], fp8, tag=f"G{d}", name=f"G{d}")
                  for d in range(DPC)]
            hidT = consts.tile([128, DPC, L], bf16)
            o_all = consts.tile([128, 2 * DPC, 8], f32)
            e_all = consts.tile([128, 2 * DPC, 8], f32)
            s7p = consts.tile([128, 2 * DPC], f32)
            ls = consts.tile([128, 2 * DPC], f32)
            lnscr = consts.tile([128, 1], f32)
            ssums = [work.tile([128, 2], f32, tag=f"ssum{d}", name=f"ssum{d}")
                     for d in range(DPC)]
            rrs = [work.tile([128, 2], f32, tag=f"rr{d}", name=f"rr{d}")
                   for d in range(DPC)]

            # zero pads / constant-one feature row (Pool)
            nc.gpsimd.memset(o2p[:, :, 1, :], 0.0)
            nc.gpsimd.memset(o2p[0:1, :, 1, :], 1.0)
            for d in range(DPC):
                nc.gpsimd.memset(XcTs[d][:, MT, :], 0.0)
            nc.gpsimd.memset(o_all, 0.0)

            def bc_ap(t, ap):
                src = t[:]
                return bass.AP(tensor=src.tensor, offset=src.offset, ap=ap)

            # warm-up: hold the PE clock ramp during the DMA lead-in.
            warm_in = consts.tile([128, 128], bf16)
            nc.vector.memset(warm_in, 0.0)
            warm = ps.tile([128, 512], f32, tag="mm")
            for _ in range(85):
                mm(warm[:, :128], lhsT=warm_in, rhs=warm_in, start=True,
                   stop=True, skip_group_check=True)

            # round-robin copy-out engine picker (psum sources: DVE/ACT only)
            rr_state = [0]

            def copy_scaled(out, in_, scale, bias_col=None, eng=None):
                """out = in_*scale (+bias_col) on DVE/ACT ('v'/'a'),
                alternating if eng is None. scale: float or [128,1] AP."""
                if eng is None:
                    eng = 'v' if rr_state[0] % 2 == 0 else 'a'
                    rr_state[0] += 1
                if eng == 'v':
                    if bias_col is None:
                        nc.vector.tensor_scalar(out=out, in0=in_, scalar1=scale,
                                                scalar2=None, op0=OP.mult)
                    else:
                        nc.vector.tensor_scalar(out=out, in0=in_, scalar1=scale,
                                                scalar2=bias_col, op0=OP.mult,
                                                op1=OP.add)
                else:
                    nc.scalar.activation(out, in_, AF.Identity, scale=scale,
                                         bias=bias_col if bias_col is not None
                                         else 0.0)

            # ---------------- stage 1: xr = x @ w_rel ----------------
            # psum = 256*xr_half; xr stores 16*xr masked by src speaker
            for i in range(NT):
                for h2 in range(2):
                    p = ps.tile([128, 512], f32, tag="mm")
                    for kk in range(KT // 2):
                        mm(p, lhsT=xt[:, 2 * kk:2 * kk + 2, i * 128:(i + 1) * 128],
                           rhs=wrel[:, 2 * kk:2 * kk + 2, h2 * 512:(h2 + 1) * 512],
                           start=(kk == 0), stop=(kk == KT // 2 - 1), perf_mode=DR)
                    copy_scaled(xr[:, i, h2 * 512:(h2 + 1) * 512], p,
                                spk[:, i, h2:h2 + 1])

            # ---------------- stage 2: out1T = 32*(agg/deg + x@w_root1 + b1)
            # mask values are 2/deg; xr already speaker-masked
            for d in range(DPC):
                pa = ps.tile([128, 512], f32, tag="mm")
                for e in range(4):          # e = (dst_spk, dir)
                    for a in range(2):      # src speaker half
                        r = a * 4 + e
                        mm(pa[:, :L],
                           lhsT=xr[:, 2 * d:2 * d + 2, r * H:(r + 1) * H],
                           rhs=at[:, d, e, :, :], start=(e == 0 and a == 0),
                           stop=False, perf_mode=DR)
                for kk in range(KT // 2):   # root term: same psum group
                    mm(pa[:, :L], lhsT=wr1[:, 2 * kk:2 * kk + 2, :],
                       rhs=xt[:, 2 * kk:2 * kk + 2, d * L:(d + 1) * L],
                       start=False, stop=(kk == KT // 2 - 1), perf_mode=DR)
                nc.vector.tensor_scalar(out=out1T[:, d, :], in0=pa[:, :L],
                                        scalar1=bias[:, 0:1], scalar2=None,
                                        op0=OP.add)

            # out1 (fp8, 2*out1) via PE transpose
            for d in range(DPC):
                for st in range(2):
                    tp = pst.tile([128, 128], bf16, tag="tr")
                    nc.tensor.transpose(tp, out1T[:, d, st * 128:(st + 1) * 128],
                                        ident)
                    nc.vector.tensor_scalar(out=out1[:, 2 * d + st, :], in0=tp,
                                            scalar1=S_O1 / S_WR1, scalar2=None,
                                            op0=OP.mult)

            # ---------------- stage 3: GraphConv layer 2 ----------------
            nbTs = []
            for d in range(DPC):
                p2 = ps.tile([128, 512], f32, tag="mm")
                mm(p2[:, :L], lhsT=out1[:, 2 * d:2 * d + 2, :], rhs=bt[:, d, :, :],
                   start=True, stop=True, perf_mode=DR)
                nbT = work.tile([128, L], bf16, tag=f"nbT{d}", name=f"nbT{d}")
                copy_scaled(nbT, p2[:, :L], 1.0)
                nbTs.append(nbT)
            for d in range(DPC):
                p3 = ps.tile([128, 512], f32, tag="mm")
                mm(p3[:, :L], lhsT=w2[:, 0, :], rhs=nbTs[d], start=True, stop=False)
                mm(p3[:, :L], lhsT=w2[:, 1, :], rhs=out1T[:, d, :],
                   start=False, stop=True)
                copy_scaled(o2p[:, d, 0, :], p3[:, :L], 1.0, bias[:, 1:2])

            # M^T k-pair accessor: j in 0..4 -> [128, 2, cols] fp8
            def mpair(j, d, c0=0, c1=L):
                if j < 4:
                    return xt[:, 2 * j:2 * j + 2, d * L + c0:d * L + c1]
                return o2p[:, d, :, c0:c1]

            # ---------------- G = M @ w_lin (independent of stage 5/6) ----
            for d in range(DPC):
                for st in range(2):
                    pg = ps.tile([128, 512], f32, tag="mm")
                    for j in range(5):
                        mm(pg[:, :H], lhsT=mpair(j, d, st * 128, (st + 1) * 128),
                           rhs=wlin[:, 2 * j:2 * j + 2, :],
                           start=(j == 0), stop=(j == 4), perf_mode=DR)
                    copy_scaled(Gs[d][:, st, :], pg[:, :H], S_G / S_WL,
                                eng='v')

            # ------------- per-dialogue pipelined back half ---------------
            def s5(d):
                # Xc^T = 2*(M @ w_t + b_t) (b_t via ones-feature, wt plane 9)
                for n2 in range(MT):
                    p4 = ps.tile([128, 512], f32, tag="mm")
                    for j in range(5):
                        mm(p4[:, :L], lhsT=wt[:, 2 * j:2 * j + 2,
                                              n2 * 128:(n2 + 1) * 128],
                           rhs=mpair(j, d), start=(j == 0), stop=(j == 4),
                           perf_mode=DR)
                    copy_scaled(XcTs[d][:, n2, :], p4[:, :L], 1.0 / S_WT,
                                eng=('a' if n2 % 3 == 2 else 'v'))

            def s6(d):
                # scores -> tanh -> exp -> normalized 64*alpha (bf16)
                for tt in range(2):
                    p5 = ps.tile([128, 512], f32, tag="mm")
                    for j in range(5):
                        mm(p5[:, :L],
                           lhsT=XcTs[d][:, 2 * j:2 * j + 2,
                                        tt * 128:(tt + 1) * 128],
                           rhs=mpair(j, d), start=(j == 0), stop=(j == 4),
                           perf_mode=DR)
                    if use_mask:
                        zf = work.tile([128, L], f32, tag="zf", name="zf")
                        nc.vector.tensor_mul(zf, p5[:, :L], um[:, d, 0, :])
                        nc.scalar.activation(zs[d][:, tt, :], zf, AF.Tanh)
                    else:
                        nc.scalar.activation(zs[d][:, tt, :], p5[:, :L], AF.Tanh,
                                             scale=1.0 / S_XC)
                nc.scalar.activation(alfs[d], zs[d], AF.Exp, bias=bias[:, 3:4])
                if use_mask:
                    for tt in range(2):
                        nc.gpsimd.tensor_mul(alfs[d][:, tt, :],
                                             alfs[d][:, tt, :], um[:, d, 1, :])
                for tt in range(2):
                    nc.vector.reduce_sum(out=ssums[d][:, tt:tt + 1],
                                         in_=alfs[d][:, tt, :], axis=AX)
                nc.vector.reciprocal(rrs[d], ssums[d])
                nc.gpsimd.tensor_scalar_mul(rrs[d], rrs[d], 64.0)
                nc.gpsimd.tensor_tensor(
                    out=alfs[d], in0=alfs[d],
                    in1=bc_ap(rrs[d], list(rrs[d][:].ap[:-1])
                              + [list(rrs[d][:].ap[-1]), [0, L]]),
                    op=OP.mult)

            def att_cls(d):
                # alpha^T (PE transpose) -> hidden^T -> logits -> o_all
                for tt in range(2):
                    for st in range(2):
                        tp = pst.tile([128, 128], bf16, tag="tr")
                        nc.tensor.transpose(
                            tp, alfs[d][:, tt, st * 128:(st + 1) * 128], ident)
                        copy_scaled(alphaTs[d][:, st, tt * 128:(tt + 1) * 128],
                                    tp, 1.0)
                p7 = ps.tile([128, 512], f32, tag="mm")
                mm(p7[:, :L], lhsT=Gs[d][:, :, :], rhs=alphaTs[d][:, :, :],
                   start=True, stop=True, perf_mode=DR)
                nc.scalar.activation(hidT[:, d, :], p7[:, :L], AF.Relu,
                                     scale=1.0 / (S_G * 64.0), bias=bias[:, 2:3])
                for tt in range(2):
                    g = 2 * d + tt
                    p8 = ps.tile([128, 512], f32, tag="mm")
                    mm(p8[:, :C], lhsT=hidT[:, d, tt * 128:(tt + 1) * 128],
                       rhs=wfc, start=True, stop=True)
                    nc.vector.tensor_tensor(out=o_all[:, g, 0:7], in0=p8[:, :C],
                                            in1=bfc[:, 0:7], op=OP.add)

            # schedule: softmax(d) latency hides under s5/s6 of d+1
            s5(0); s6(0)
            s5(1); s6(1)
            att_cls(0)
            s5(2); s6(2)
            att_cls(1)
            s5(3); s6(3)
            # preload the natural_log_exp ACT table (covers exp/relu/identity
            # for everything after the last tanh; Ln then loads no table)
            nc.scalar.activation(lnscr, bias[:, 3:4], AF.Ln)
            att_cls(2)
            att_cls(3)

            # log_softmax over C=7 (logits are tiny: skip max-sub)
            nc.scalar.activation(e_all, o_all, AF.Exp)
            nc.vector.reduce_sum(out=s7p, in_=e_all[:, :, 0:7], axis=AX)
            nc.scalar.activation(ls, s7p, AF.Ln)
            nc.vector.tensor_tensor(
                out=o_all[:, :, 0:7], in0=o_all[:, :, 0:7],
                in1=bc_ap(ls, [list(ls[:].ap[0]), list(ls[:].ap[1]), [0, 7]]),
                op=OP.subtract)
            dma_sp(out=out_d[:].rearrange("(g p) c -> p g c", p=128),
                   in_=o_all[:, :, 0:7])

    return _finish(nc)


def _finish(nc):
    nc.compile()
    return nc


def prep_inputs(x, edge_src, edge_dst, edge_type, umask, basis, comp,
                w_root1, b1, w_rel2, b_rel2, w_root2, w_t, b_t,
                w_lin, b_lin, w_fc, b_fc):
    """Host-side sharding / layout / fp8-scale prep."""
    x = np.asarray(x, np.float32)
    src = np.asarray(edge_src, np.int64)
    dst = np.asarray(edge_dst, np.int64)
    ety = np.asarray(edge_type, np.int64)
    umask = np.asarray(umask, np.float32)

    g_s = src // L
    assert np.array_equal(g_s, dst // L), "edges must stay within a dialogue"

    def q8(a, scale):
        return np.clip(np.asarray(a, np.float32) * scale,
                       -240.0, 240.0).astype(FP8)

    w_rel = np.einsum('rb,bdh->rdh', np.asarray(comp, np.float32),
                      np.asarray(basis, np.float32))
    wrel_in = q8(np.ascontiguousarray(
        w_rel.transpose(1, 0, 2).reshape(D, R * H)), S_WREL)

    deg = np.bincount(dst, minlength=N).astype(np.float64)
    inv2 = (2.0 / np.maximum(deg, 1)) * (deg > 0)

    # speaker of each node from edge types (etype = a*4 + b*2 + dir)
    spk_node = np.zeros(N, np.int64)
    spk_node[src] = ety // 4
    spk_in = np.zeros((N, 2), np.float32)
    spk_in[np.arange(N), spk_node] = 1.0 / S_XR

    # (b, dir) masks with 2/deg baked into the values
    at_all = np.zeros((B, 4, L, L), np.float32)
    np.add.at(at_all, (g_s, ety % 4, src % L, dst % L), inv2[dst])
    bt_all = np.zeros((B, L, L), np.float32)
    np.add.at(bt_all, (g_s, src % L, dst % L), 1.0)
    # p-major layouts with >=512B contiguous inner runs
    at_all = at_all.reshape(B, 4, 2, 128, L).transpose(0, 1, 3, 2, 4)
    bt_all = bt_all.reshape(B, 2, 128, L).transpose(0, 2, 1, 3)

    use_mask = not bool(np.all(umask == 1.0))

    wt_in = np.zeros((MTP * 128, MEM), np.float32)
    wt_in[:MEM] = np.asarray(w_t, np.float32) * S_WT
    wt_in[MEM] = np.asarray(b_t, np.float32) * S_WT    # ones-feature row
    wlin_in = np.zeros((MTP * 128, H), np.float32)
    wlin_in[:MEM] = np.asarray(w_lin, np.float32) * S_WL

    bias_pack = np.zeros((128, 4), np.float32)
    bias_pack[:, 0] = np.asarray(b1, np.float32) * S_WR1
    bias_pack[:, 1] = np.asarray(b_rel2, np.float32)
    bias_pack[:, 2] = np.asarray(b_lin, np.float32)
    bias_pack[:, 3] = math.log(S_AL)

    bfc_in = np.zeros((1, 8), np.float32)
    bfc_in[0, :C] = np.asarray(b_fc, np.float32)

    shared = {
        "wrel": wrel_in,
        "wr1": q8(w_root1, S_WR1),
        "w2": np.stack([np.asarray(w_rel2, np.float32) / S_O1,
                        np.asarray(w_root2, np.float32) / S_WR1]).astype(BF16),
        "wt": np.clip(wt_in, -240, 240).astype(FP8),
        "wlin": np.clip(wlin_in, -240, 240).astype(FP8),
        "wfc": np.asarray(w_fc, np.float32).astype(BF16),
        "bias": bias_pack,
        "bfc": bfc_in,
    }

    in_maps = []
    for c in range(NCORES):
        xl = x[c * NLOC:(c + 1) * NLOC]
        m = dict(shared)
        m["xt"] = q8(np.ascontiguousarray(xl.T), 1.0)
        m["spk"] = spk_in[c * NLOC:(c + 1) * NLOC]
        m["at"] = at_all[c * DPC:(c + 1) * DPC].astype(FP8)
        m["bt"] = bt_all[c * DPC:(c + 1) * DPC].astype(FP8)
        if use_mask:
            uml = umask[c * DPC:(c + 1) * DPC]   # (DPC, L)
            m["um"] = np.stack([uml * uml / S_XC, uml], axis=1
                               ).astype(np.float32)
        in_maps.append(m)
    return in_maps, use_mask


_last_results = None


def kernel(**inputs):
    global _last_results
    from concourse.bass_utils import run_bass_kernel_spmd

    in_maps, use_mask = prep_inputs(**inputs)
    if use_mask not in _cache:
        _cache[use_mask] = _build_program(use_mask)
    nc = _cache[use_mask]
    res = run_bass_kernel_spmd(nc, in_maps, core_ids=list(range(NCORES)))
    _last_results = res
    return np.concatenate([res.results[c]["out"] for c in range(NCORES)],
                          axis=0)
